# revision 61
# baseline (speedup 1.0000x reference)
"""Trainium2 Bass kernel for nn_Net_17532056502451.

5 "think" iterations: shift-window cosine selector (159 shifts) + softmax
attention + scatter-back + conv-style encoder/decoder with energy argmax
(81 shifts), masked-MSE losses averaged.  Data-parallel: 1024 tokens over
8 cores, 128 tokens/core (one per SBUF partition), token-major fp32.

Mappings per core (1.45 ms -> 0.268 ms over the session):
- dot correlation: one fp16 3D tensor_tensor (overlapping-window AP x
  broadcast AP, DVE 2x packed mode) + in-place tree-adds + a final
  5-wide reduce.  fp16 product/partial rounding, fp32 internal accum.
- sliding norms: Square + prefix-scan + strided diff (fp32, exact).
- argmaxes: nc.vector.max / max_index (first-occurrence ties = jnp.argmax).
- per-token dynamic windows (yal/xele/yhat/yele): DVE barrel shifter --
  log2 stages of in-place forward copy_predicated; stage masks come from
  the u32 argmax index via one bitwise_and against (1<<k) rows; derived
  offsets (159-theta, 80-s*) computed as (~i & mask) - const to dodge
  u32 saturation.  Exact (pure data movement).
- energy: E[t,s] = ye^T C[dd:,dd:] ye + 2 q[dd:]^T ye (+bb dropped;
  argmax-invariant), via host-precomputed Cslide (80b x 81s*88a bf16):
  M = yeT16 @ Cslide in 14 PE matmuls (PSUM->SBUF copies mostly ACT),
  with E = sum_a M[t,88s+a]*[ye;1] computed in 4 s-sliced fp16
  mult/tree/reduce groups interleaved into the chunk loop so DVE
  overlaps the matmul window; x_ele gather + next-iter norms run in
  the decoder window.
- decoder: h never materialized -- x_ext = [F; g]^T @ [yhat; 1] with
  F = W_src @ W_enc (160x160 fp32), bias row folded into the stationary:
  2 accumulating matmuls straight to token-major, no back-transposes.
"""
import numpy as np

IDIM = 80
ODIM = 80
HDIM = 512
THINK_ITER = 5
TEMPER = 0.7
B, T = 4, 256
NTOK = B * T
P = 128
NCORES = 8
S1 = 159
S2 = 81
NFEAT = 80 * 80
NCHUNK = NFEAT // 128   # 50

_cache = {}


def _build_consts(W_enc, b_enc, W_src, b_src):
    W_enc = np.asarray(W_enc, np.float32)
    b_enc = np.asarray(b_enc, np.float32)
    W_src = np.asarray(W_src, np.float32)
    b_src = np.asarray(b_src, np.float32)
    C = (W_enc.T @ W_enc).astype(np.float32)
    q = (W_enc.T @ b_enc).astype(np.float32)
    # E[t,s] = sum_{a,b<80} C[dd+a, dd+b] ye_a ye_b + sum_b 2 q[dd+b] ye_b
    # (+ const bb, dropped: argmax-invariant), dd = 80 - s.
    # Cslide[b, 81a + s] = C[a+dd, b+dd] (a<80); row a=80 carries the
    # linear term 2 q[dd+b]; consumed as E = sum_a M[t,(a,s)] * ye1[a],
    # M = Cslide^T-contracted with ye over b, ye1 = [ye; 1].
    import ml_dtypes
    dd = 80 - np.arange(S2)                           # (81,)
    a_i = np.arange(80)
    b_i = np.arange(80)
    A3 = C[(a_i[:, None, None] + dd[None, :, None]),
           (b_i[None, None, :] + dd[None, :, None])]  # (a, s, b)
    lin = 2.0 * q[dd[:, None] + b_i[None, :]]         # (s, b)
    # s-major, a padded to 88 (even strides for DVE 2x; tree 88->44->22->11)
    Cs3 = np.zeros((80, 81, 88), np.float32)          # (b, s, a)
    Cs3[:, :, 0:80] = A3.transpose(2, 1, 0)
    Cs3[:, :, 80] = lin.T
    Cs16 = Cs3.reshape(80, 81 * 88).astype(ml_dtypes.bfloat16)
    # fused decoder: x_ext = F @ yhat + g, F = W_src @ W_enc (160, 160)
    F = (W_src @ W_enc).astype(np.float32)
    FT = np.ascontiguousarray(F.T)                    # (160c, 160o)
    g = (W_src @ b_enc + b_src).astype(np.float32)    # (160,)
    F1g = np.vstack([FT[128:160], g[None, :]])        # (33, 160): bias row
    ident = np.eye(128, dtype=np.float32)
    ident16 = np.eye(128, dtype=ml_dtypes.bfloat16)
    mrow = np.broadcast_to(
        (1 << np.arange(8, dtype=np.uint32)), (P, 8)).copy()  # (128, 8)
    return dict(Cs=Cs16, FT=FT, F1g=F1g,
                mrow=mrow, ident=ident, ident16=ident16)


def _build_nc():
    import concourse.bass as bass
    import concourse.bacc as bacc
    import concourse.mybir as mybir
    from concourse.tile import TileContext

    F32 = mybir.dt.float32
    BF16 = mybir.dt.bfloat16
    U32 = mybir.dt.uint32
    Op = mybir.AluOpType
    AF = mybir.ActivationFunctionType

    nc = bacc.Bacc()
    d_x = nc.declare_dram_parameter("x", [P, 80], F32, isOutput=False)
    d_y = nc.declare_dram_parameter("y", [P, 80], F32, isOutput=False)
    d_Cs = nc.declare_dram_parameter("Cs", [80, 81 * 88], BF16, isOutput=False)
    d_F = nc.declare_dram_parameter("FT", [160, 160], F32, isOutput=False)
    d_F1g = nc.declare_dram_parameter("F1g", [33, 160], F32, isOutput=False)
    d_mr = nc.declare_dram_parameter("mrow", [P, 8], U32, isOutput=False)
    d_id = nc.declare_dram_parameter("ident", [128, 128], F32, isOutput=False)
    d_id16 = nc.declare_dram_parameter("ident16", [128, 128], BF16, isOutput=False)
    d_out = nc.declare_dram_parameter("losspart", [P, 8], F32, isOutput=True)

    with TileContext(nc) as tc:
        with (
            tc.tile_pool(name="const", bufs=1) as cpool,
            tc.tile_pool(name="work", bufs=1) as pool,
            tc.tile_pool(name="ps_rot", bufs=3, space="PSUM") as pp,
            tc.tile_pool(name="ps_he", bufs=2, space="PSUM") as pph,
            tc.tile_pool(name="ps_ye", bufs=1, space="PSUM") as ppy,
        ):
            # ---- inputs first: x/y DMA ahead of bulky consts ----
            xpad = pool.tile([P, 335], F32, tag="xpad")
            yres = pool.tile([P, 80], F32, tag="yres")
            nc.vector.memset(xpad[:], 0.0)
            nc.sync.dma_start(xpad[:, 79:159], d_x[:])
            nc.sync.dma_start(yres[:], d_y[:])
            # ---- constants ----
            Cs_t = cpool.tile([80, 81 * 88], BF16, tag="Cs")
            nc.sync.dma_start(Cs_t[:], d_Cs[:])
            F0_t = cpool.tile([128, 160], F32, tag="F0")
            nc.sync.dma_start(F0_t[:], d_F[0:128, :])
            F1g_t = cpool.tile([33, 160], F32, tag="F1g")
            nc.sync.dma_start(F1g_t[:], d_F1g[:])
            mr_t = cpool.tile([P, 8], U32, tag="mr")
            nc.sync.dma_start(mr_t[:], d_mr[:])
            id_t = cpool.tile([128, 128], F32, tag="id")
            nc.sync.dma_start(id_t[:], d_id[:])
            id16_t = cpool.tile([128, 128], BF16, tag="id16")
            nc.sync.dma_start(id16_t[:], d_id16[:])

            # ---- state ----
            keep = pool.tile([P, 80], F32, tag="keep")
            yap = pool.tile([P, 335], F32, tag="yap")
            lossp = pool.tile([P, 8], F32, tag="lossp")
            nc.vector.memset(yap[:], 0.0)
            nc.vector.memset(lossp[:], 0.0)
            nc.vector.tensor_scalar(keep[:], yres[:], 0.0, None, Op.not_equal)

            sqx = pool.tile([P, 239], F32, tag="sqx")
            nc.vector.memset(sqx[:, 0:1], 0.0)
            cs = pool.tile([P, 239], F32, tag="cs")
            nsq = pool.tile([P, S1], F32, tag="nsq")
            adot = pool.tile([P, S1], F32, tag="adot")
            gsel = pool.tile([P, S1], F32, tag="gsel")
            rnsq = pool.tile([P, S1], F32, tag="rnsq")
            mx8 = pool.tile([P, 8], F32, tag="mx8")
            mi8 = pool.tile([P, 8], U32, tag="mi8")
            t2 = pool.tile([P, 1], U32, tag="t2")
            d4 = pool.tile([P, 1], U32, tag="d4")
            m8a = pool.tile([P, 8], U32, tag="m8a")
            m8b = pool.tile([P, 8], U32, tag="m8b")
            m8c = pool.tile([P, 8], U32, tag="m8c")
            m8d = pool.tile([P, 8], U32, tag="m8d")
            bbYal = pool.tile([P, 208], F32, tag="bbYal")
            bbXele = pool.tile([P, 208], F32, tag="bbXele")
            bbYhat = pool.tile([P, 224], F32, tag="bbYhat")
            bbYele = pool.tile([P, 144], F32, tag="bbYele")
            yal = bbYal[:, 0:80]
            xele = bbXele[:, 0:80]
            yhat = bbYhat[:, 0:160]
            yele = bbYele[:, 0:80]
            zt = pool.tile([P, 80], F32, tag="zt")
            et = pool.tile([P, 80], F32, tag="et")
            ssum = pool.tile([P, 1], F32, tag="ssum")
            rsum = pool.tile([P, 1], F32, tag="rsum")
            nzm = pool.tile([P, 1], F32, tag="nzm")
            zero1 = pool.tile([P, 1], F32, tag="zero1")
            nc.vector.memset(zero1[:], 0.0)
            ye16 = pool.tile([P, 80], BF16, tag="ye16")
            yeT16 = pool.tile([80, 128], BF16, tag="yeT16")
            F16 = mybir.dt.float16
            xpad16 = pool.tile([P, 238], F16, tag="xpad16")
            yres16 = pool.tile([P, 80], F16, tag="yres16")
            q3dot = pool.tile([P, S1 * 80], F16, tag="q3dot")
            dot16 = pool.tile([P, S1], F16, tag="dot16")
            ye116 = pool.tile([P, 88], F16, tag="ye116")
            nc.vector.memset(ye116[:, 80:81], 1.0)
            nc.vector.memset(ye116[:, 81:88], 0.0)
            M16 = pool.tile([P, 81 * 88], F16, tag="M16")
            q16 = pool.tile([P, 81 * 88], F16, tag="q16")
            Etok16 = pool.tile([P, S2], F16, tag="Etok16")
            mx8h = pool.tile([P, 8], F16, tag="mx8h")
            yhT0 = pool.tile([128, 128], F32, tag="yhT0")
            yhT1 = pool.tile([33, 128], F32, tag="yhT1")
            nc.vector.memset(yhT1[32:33, :], 1.0)
            xext = pool.tile([P, 208], F32, tag="xext")
            nc.vector.memset(xext[:, 160:208], 0.0)
            dtmp = pool.tile([P, 80], F32, tag="dtmp")


            def barrel(src_pad, m8, buf, out_w, nbits):
                """buf[p, 0:out_w] = src_pad[p, off_p : off_p + out_w] where
                off_p's bit-k mask is m8[:, k] (nonzero when bit set)."""
                k = nbits - 1
                w = out_w + (1 << k) - 1
                nc.scalar.copy(buf[:, 0:w], src_pad[:, 0:w])
                nc.vector.copy_predicated(
                    buf[:, 0:w], m8[:, k:k + 1].to_broadcast((P, w)),
                    src_pad[:, (1 << k):(1 << k) + w])
                for k in range(nbits - 2, -1, -1):
                    w = out_w + (1 << k) - 1
                    nc.vector.copy_predicated(
                        buf[:, 0:w], m8[:, k:k + 1].to_broadcast((P, w)),
                        buf[:, (1 << k):(1 << k) + w])

            def sliding_norms():
                nc.scalar.activation(sqx[:, 1:239], xpad[:, 0:238], AF.Square)
                nc.vector.tensor_tensor_scan(cs[:], sqx[:],
                                             zero1[:].to_broadcast((P, 239)),
                                             0.0, Op.add, Op.bypass)
                nc.vector.tensor_tensor(nsq[:], cs[:, 80:239], cs[:, 0:159],
                                        Op.subtract)

            sliding_norms()
            nc.scalar.copy(xpad16[:], xpad[:, 0:238])
            nc.scalar.copy(yres16[:], yres[:])
            for it in range(THINK_ITER):
                # --- dot[t,s] = sum_c xpad[t,s+c] yres[t,c]: fp16 3D ---
                in0d = bass.AP(xpad16.tensor, xpad16[:].offset,
                               [list(xpad16[:].ap[0]), [1, S1], [1, 80]])
                in1d = bass.AP(yres16.tensor, yres16[:].offset,
                               [list(yres16[:].ap[0]), [0, S1], [1, 80]])
                qd = q3dot[:].rearrange("p (s c) -> p s c", c=80)
                nc.vector.tensor_tensor(qd, in1d, in0d, Op.mult)
                # tree-add over c: 80 -> 40 -> 20 -> 10 -> 5, then reduce 5
                for hw_ in (40, 20, 10, 5):
                    nc.vector.tensor_tensor(qd[:, :, 0:hw_], qd[:, :, 0:hw_],
                                            qd[:, :, hw_:2 * hw_], Op.add)
                with nc.allow_low_precision(reason="fp16 dot, fp32 internal"):
                    nc.vector.tensor_reduce(dot16[:], qd[:, :, 0:5],
                                            mybir.AxisListType.X, Op.add)
                # --- theta = argmax dot*|dot|/nsq ---
                nc.scalar.activation(adot[:], dot16[:], AF.Abs)
                nc.vector.tensor_scalar_max(rnsq[:], nsq[:], 1e-30)
                nc.vector.reciprocal(rnsq[:], rnsq[:])
                nc.vector.tensor_tensor(gsel[:], dot16[:], adot[:], Op.mult)
                nc.vector.tensor_tensor(gsel[:], gsel[:], rnsq[:], Op.mult)
                nc.vector.max(mx8[:], gsel[:])
                nc.vector.max_index(mi8[:], mx8[:], gsel[:])
                # --- masks: theta bits, (159 - theta) bits ---
                nc.vector.tensor_tensor(m8a[:], mi8[:, 0:1].to_broadcast((P, 8)),
                                        mr_t[:], Op.bitwise_and)
                if it + 1 < THINK_ITER:
                    # 159 - theta == (~theta & 255) - 96 (u32 saturation-safe)
                    nc.vector.tensor_scalar(t2[:], mi8[:, 0:1], 0, 255,
                                            Op.bitwise_not, Op.bitwise_and)
                    nc.vector.tensor_scalar(t2[:], t2[:], 96, None, Op.subtract)
                    nc.vector.tensor_tensor(m8b[:], t2[:].to_broadcast((P, 8)),
                                            mr_t[:], Op.bitwise_and)
                # --- y_align gather: yal[j] = xpad[theta + j] ---
                barrel(xpad, m8a, bbYal, 80, 8)
                # --- softmax attention -> y_att in yap[:, 80:160] ---
                nc.vector.tensor_tensor(zt[:], yal, yres[:], Op.mult)
                nc.vector.max(mx8[:], zt[:])
                nc.vector.tensor_scalar_mul(nzm[:], mx8[:, 0:1], -1.0 / TEMPER)
                nc.scalar.activation(et[:], zt[:], AF.Exp, bias=nzm[:, 0:1],
                                     scale=1.0 / TEMPER)
                nc.vector.tensor_reduce(ssum[:], et[:], mybir.AxisListType.X, Op.add)
                nc.vector.reciprocal(rsum[:], ssum[:])
                nc.vector.tensor_tensor(et[:], et[:], yal, Op.mult)
                nc.vector.tensor_scalar_mul(yap[:, 80:160], et[:], rsum[:, 0:1])
                # --- E via Cslide: M[t, 81a+s] = sum_b Cs[b, 81a+s] ye[t,b] ---
                nc.scalar.copy(ye16[:], yap[:, 80:160])
                nc.scalar.copy(ye116[:, 0:80], yap[:, 80:160])
                yeTp = ppy.tile([128, 128], BF16, tag="yeTp")
                nc.tensor.transpose(yeTp[0:80, :], ye16[:], id16_t[:])
                nc.scalar.copy(yeT16[:], yeTp[0:80, :])
                qv = q16[:].rearrange("p (s ar) -> p s ar", ar=88)

                def eslice(s0, s1):
                    ns = s1 - s0
                    qs = qv[:, s0:s1, :]
                    i0 = bass.AP(ye116.tensor, ye116[:].offset,
                                 [list(ye116[:].ap[0]), [0, ns], [1, 88]])
                    i1 = bass.AP(M16.tensor, M16[:].offset + 88 * s0,
                                 [list(M16[:].ap[0]), [88, ns], [1, 88]])
                    nc.vector.tensor_tensor(qs, i0, i1, Op.mult)
                    for hw_ in (44, 22, 11):
                        nc.vector.tensor_tensor(qs[:, :, 0:hw_],
                                                qs[:, :, 0:hw_],
                                                qs[:, :, hw_:2 * hw_], Op.add)
                    with nc.allow_low_precision(reason="fp16 E"):
                        nc.vector.tensor_reduce(Etok16[:, s0:s1],
                                                qs[:, :, 0:11],
                                                mybir.AxisListType.X, Op.add)

                ebounds = {3: (0, 20), 6: (20, 40), 10: (40, 60), 13: (60, 81)}
                for k in range(14):
                    c0 = k * 512
                    cw = min(512, 81 * 88 - c0)
                    Mp = pp.tile([128, 512], F32, tag="Mp")
                    nc.tensor.matmul(Mp[:, 0:cw], yeT16[:],
                                     Cs_t[:, c0:c0 + cw],
                                     start=True, stop=True)
                    if k % 4 == 3:
                        nc.vector.tensor_copy(M16[:, c0:c0 + cw], Mp[:, 0:cw])
                    else:
                        nc.scalar.copy(M16[:, c0:c0 + cw], Mp[:, 0:cw])
                    if k in ebounds:
                        eslice(*ebounds[k])
                # --- s* argmax; masks for s* and d* = 80 - s* ---
                nc.vector.max(mx8h[:], Etok16[:])
                nc.vector.max_index(mi8[:], mx8h[:], Etok16[:])
                nc.vector.tensor_tensor(m8c[:], mi8[:, 0:1].to_broadcast((P, 8)),
                                        mr_t[:], Op.bitwise_and)
                # 80 - s* == (~s* & 127) - 47, avoids u32 saturation
                nc.vector.tensor_scalar(d4[:], mi8[:, 0:1], 0, 127,
                                        Op.bitwise_not, Op.bitwise_and)
                nc.vector.tensor_scalar(d4[:], d4[:], 47, None, Op.subtract)
                nc.vector.tensor_tensor(m8d[:], d4[:].to_broadcast((P, 8)),
                                        mr_t[:], Op.bitwise_and)
                # --- yhat embed: yhat[j] = yap[s* + j], j in [0,160) ---
                barrel(yap, m8c, bbYhat, 160, 7)
                # --- x_ele gather + next-iter prep (overlaps decoder PE/ACT) ---
                if it + 1 < THINK_ITER:
                    barrel(yap, m8b, bbXele, 80, 8)
                    nc.vector.tensor_tensor(xpad[:, 79:159], xpad[:, 79:159],
                                            xele, Op.subtract)
                    sliding_norms()
                    nc.scalar.copy(xpad16[:], xpad[:, 0:238])
                # --- x_extT = F @ yhat^T (+ g), F = W_src W_enc fused ---
                yhTp = pph.tile([128, 128], F32, tag="yaTp")
                nc.tensor.transpose(yhTp[:], bbYhat[:, 0:128], id_t[:])
                nc.scalar.copy(yhT0[:], yhTp[:])
                yhTp2 = pph.tile([128, 128], F32, tag="yaTp")
                nc.tensor.transpose(yhTp2[0:32, :], bbYhat[:, 128:160], id_t[:])
                nc.scalar.copy(yhT1[0:32, :], yhTp2[0:32, :])
                Xp = pph.tile([128, 160], F32, tag="Hp")
                nc.tensor.matmul(Xp[:], yhT0[:], F0_t[:],
                                 start=True, stop=False)
                nc.tensor.matmul(Xp[:], yhT1[:], F1g_t[:],
                                 start=False, stop=True)
                nc.scalar.copy(xext[:, 0:160], Xp[:])
                # --- y_ele gather: yele[j] = xext[d* + j] ---
                barrel(xext, m8d, bbYele, 80, 7)
                # --- loss partial + state updates ---
                nc.vector.tensor_tensor(dtmp[:], yele, yres[:], Op.subtract)
                nc.vector.tensor_tensor(dtmp[:], dtmp[:], keep[:], Op.mult)
                nc.scalar.activation(et[:], dtmp[:], AF.Square)
                nc.vector.tensor_reduce(lossp[:, it:it + 1], et[:],
                                        mybir.AxisListType.X, Op.add)
                if it + 1 < THINK_ITER:
                    nc.vector.tensor_tensor(yres[:], yres[:], yele, Op.subtract)
                    nc.scalar.copy(yres16[:], yres[:])

            nc.sync.dma_start(d_out[:], lossp[:])
    return nc


def kernel(x, y, W_enc, b_enc, W_src, b_src):
    import sys
    if '/opt/trn_rl_repo' not in sys.path:
        sys.path.insert(0, '/opt/trn_rl_repo')
    x = np.asarray(x, np.float32)
    y = np.asarray(y, np.float32)
    consts = _build_consts(W_enc, b_enc, W_src, b_src)

    if "nc" not in _cache:
        _cache["nc"] = _build_nc()
        _cache["nc"].finalize()
    nc = _cache["nc"]

    xt = x.reshape(NTOK, IDIM)
    yt = y.reshape(NTOK, ODIM)
    in_maps = []
    for c in range(NCORES):
        m = dict(consts)
        m["x"] = np.ascontiguousarray(xt[c * P:(c + 1) * P])
        m["y"] = np.ascontiguousarray(yt[c * P:(c + 1) * P])
        in_maps.append(m)

    from concourse.bass_utils import run_bass_kernel_spmd
    res = run_bass_kernel_spmd(nc, in_maps, list(range(NCORES)))
    parts = np.stack([r["losspart"] for r in res.results])
    keep_cnt = max(int((y != 0.0).sum()), 1)
    nums = parts[:, :, :THINK_ITER].sum(axis=(0, 1), dtype=np.float64)
    losses = (nums / keep_cnt).astype(np.float32)
    return np.float32(np.mean(losses))
